# revision 76
# baseline (speedup 1.0000x reference)
"""Causal multi-head attention on 8 trn2 NeuronCores.

Sharding: head-parallel. Each core owns 2 of the 16 heads (128 of the 1024
channels) for all 4 batches. Per core:
  Q^T/K^T/V^T projections (local 128 channels) from x^T (host-transposed,
  a pure layout prep like the weight slicing); flash-style causal attention
  in score-transposed layout S^T[k, q]; softmax denominators ride along as a
  ones column appended to V (PV matmul M=65/66, den lands on its own PSUM
  partition); normalization is applied to A^T via a rank-2 "R" matmul built
  from the reciprocals; local Wo row-block matmul produces a full
  [8192, 1024] partial per core, summed (+bias) on host.

Optimizations vs the f32r baseline (470us -> ~350us on HW):
  - All heavy matmuls in bf16 (host quantizes x/W; PSUM->SBUF copies cast).
    The baseline's float32r matmuls ran the PE in its HIGH-power fp32 mode,
    which tripped hardware duty-cycle throttling (HAM k=4/8 for 38% of the
    run; bf16 halves that). bf16 also halves the x / out DMA bytes.
  - Causal masking for diagonal blocks is a post-exp multiply by a 0/1
    triangle on the (otherwise idle) Pool engine, SBUF-only (GPSIMD cannot
    touch PSUM), freeing ~60us of DVE time.
  - The softmax-reciprocal repartition uses tiny PE transposes ([2,512] den
    rows -> [128,8] -> reciprocal -> back) instead of 16 tiny DMAs per
    q-tile; the baseline's 256 DMAs cost ~650ns of serial sync-queue issue
    each (166us total).
  - x^T and the QKV weights are pre-tiled on the host so every DMA is one
    contiguous span per partition: one dma_start per (batch, q-tile) at
    ~128 descriptors instead of 8 dma_starts x 1024 descriptors. Output
    DMA is one per q-tile group. Sync-engine time: 307us -> ~40us.
  - The attention kb loop is software-pipelined one deep: scores(kb+1) and
    PE filler work are emitted between exp(kb) and PV(kb), so the FIFO PE
    queue holds runnable work while the ACT exp / Pool mask run (PV at the
    head otherwise head-of-line blocks, and every PE idle gap also drops
    the PE out of its full-clock p-state).
  - The filler queue (projections of batch b+1, Wo of batch b) backlogs
    across batches with watermark force-drains for the data deadlines, so
    the last batch's attention still has PE work to interleave.
Softmax skips the max-subtraction (scores are bounded; fp32 exp cannot
overflow) and folds the 1/sqrt(64) scale into the ACT exp. Off-diagonal
causal key blocks are skipped entirely; straddle blocks only compute/exp
their valid columns. Final: HW exec ~349-355us, rel err 4.4e-3 (gate 2e-2).
"""
import sys

sys.path.insert(0, "/opt/trn_rl_repo")

import numpy as np

import concourse.bass as bass
import concourse.tile as tile
from concourse import bacc, mybir
from concourse.bass_utils import run_bass_kernel_spmd

f32 = mybir.dt.float32
f32r = mybir.dt.float32r
bf16 = mybir.dt.bfloat16
EXP = mybir.ActivationFunctionType.Exp

B, S, D, H, HD = 4, 2048, 1024, 16, 64
NCORES = 8
CLOC = D // NCORES       # 128 local channels = 2 heads per core
BS = B * S               # 8192
QT = 4                   # q tiles of 512 per batch
KB = 16                  # k blocks of 128 per batch


def build_program():
    """Build + compile the per-core Bacc program (identical on all cores)."""
    nc = bacc.Bacc("TRN2", target_bir_lowering=False, debug=False)

    # x^T pre-tiled on host: [128 p, (b, qt, dc, s)] so each (b, qt) DMA
    # reads one contiguous 8KB span per partition (few descriptors)
    xtr_d = nc.dram_tensor("xtr", [128, B * QT * 8 * 512], bf16,
                           kind="ExternalInput").ap()
    # qkv weights pre-tiled on host into the SBUF layout [p, (c, m)]
    wq_d = nc.dram_tensor("wq", [128, D], bf16, kind="ExternalInput").ap()
    wk_d = nc.dram_tensor("wk", [128, D], bf16, kind="ExternalInput").ap()
    wv_d = nc.dram_tensor("wv", [128, D], bf16, kind="ExternalInput").ap()
    wo_d = nc.dram_tensor("wo", [CLOC, D], bf16, kind="ExternalInput").ap()
    selc_d = nc.dram_tensor("selc", [4, CLOC], f32, kind="ExternalInput").ap()
    out_d = nc.dram_tensor("out", [BS, D], bf16, kind="ExternalOutput").ap()

    with tile.TileContext(nc) as tc:
        _Builder(nc, tc, xtr_d, wq_d, wk_d, wv_d, wo_d, selc_d, out_d).build()
    nc.compile()
    return nc


class _Builder:
    def __init__(self, nc, tc, xtr_d, wq_d, wk_d, wv_d, wo_d, selc_d, out_d):
        self.nc = nc
        self.tc = tc
        self.xtr_d = xtr_d
        self.w_d = {"q": wq_d, "k": wk_d, "v": wv_d}
        self.wo_d = wo_d
        self.selc_d = selc_d
        self.out_d = out_d
        self.st_b = {}   # per-batch state: xt, qT, kT, vT, aT, v_tiles
        from collections import deque
        self.fillers = deque()
        self.n_enq = 0       # total fillers ever enqueued
        self.n_drained = 0   # total fillers ever drained
        self.markers = {}    # key -> n_enq watermark that must be drained

    def build(self):
        from contextlib import ExitStack

        nc, tc = self.nc, self.tc
        with ExitStack() as ctx:
            p = self.p = {}
            for name, bufs, space in (
                ("consts", 1, None), ("wpool", 1, None), ("xtp", 1, None),
                ("qkv", 2, None), ("vtpool", 1, None), ("vpp", 24, None),
                ("ptp", 5, None), ("atp", 2, None), ("denp", 3, None),
                ("outp", 2, None),
                ("ps_a", 2, "PSUM"), ("ps_st", 2, "PSUM"),
                ("ps_pv", 2, "PSUM"),
            ):
                kw = {"space": space} if space else {}
                p[name] = ctx.enter_context(
                    tc.tile_pool(name=name, bufs=bufs, **kw))

            self._consts()

            # ---- software pipeline across batches: proj(b+1)/Wo(b)
            # queue as PE "filler" thunks drained inside the attention
            # kb loop so the PE queue never idles on exp waits. The queue
            # deliberately backlogs across batches (no per-batch flush) so
            # the last batch's attention still has PE filler work; marker
            # force-drains enforce the data deadlines. ----
            # DMA issue order matters for startup latency (sync-queue
            # issue is serial): first q-tile of x, then the QKV weights,
            # then the rest of x, then Wo.
            self._xt_dma(0, qts=(0,))
            self._weights_qkv()
            self._xt_dma(0, qts=(1, 2, 3))
            self._weights_wo()
            self._consts_dma()
            for qt in range(QT):
                self._proj_group(0, qt)
            self._vtrans(0)
            # pend = previous q-tile awaiting its den chain; its den_part1
            # is emitted AFTER the next q-tile's scores(0)/exp(0) prologue
            # so the PE crosses the boundary with attention work in the
            # FIFO instead of den transposes blocked on DVE copies
            pend = None
            for b in range(B):
                if b + 1 < B:
                    self._xt_dma(b + 1)
                    for qt in range(QT):
                        self._enqueue_proj(b + 1, qt)
                        self.markers[("proj", b + 1, qt)] = self.n_enq
                for qt in range(QT):
                    if b >= 1:
                        self._drain_to(self.markers[("proj", b, qt)])
                    if b >= 2 and qt == 0:
                        self._drain_to(self.markers[("wo", b - 2)])
                    pro = self._attn_prologue(b, qt)
                    if pend is not None:
                        self._den_part1(*pend)
                    self._attention_qtile(b, qt, pro)
                    if pend is not None:
                        self._den_part2(*pend)
                        self._enqueue_wo(*pend)
                        if pend[1] == QT - 1:
                            self.markers[("wo", pend[0])] = self.n_enq
                    pend = (b, qt)
            self._den_part1(*pend)
            self._den_part2(*pend)
            self._enqueue_wo(*pend)
            self._drain_fillers()

    # ------------------------------------------------------------------
    def _consts(self):
        nc, p = self.nc, self.p
        # bf16 identity for the V PE-transpose
        ident = p["consts"].tile([128, 128], bf16)
        nc.gpsimd.memset(ident[:], 0.0)
        nc.gpsimd.affine_select(
            out=ident[:], in_=ident[:],
            compare_op=mybir.AluOpType.not_equal, fill=1.0, base=0,
            pattern=[[-1, 128]], channel_multiplier=1,
        )
        # 0/1 lower-triangle (valid where key p <= query col) for the
        # post-exp diagonal-block mask multiply on Pool
        trimask = p["consts"].tile([128, 128], bf16)
        nc.gpsimd.memset(trimask[:], 1.0)
        nc.gpsimd.affine_select(
            out=trimask[:], in_=trimask[:],
            compare_op=mybir.AluOpType.is_ge, fill=0.0, base=0,
            pattern=[[1, 128]], channel_multiplier=-1,
        )
        self.ident, self.trimask = ident, trimask

    def _consts_dma(self):
        # selc rows 0:2 = head-A/head-B one-hots; rows 2:4 = 2x2 identity.
        # Issued after the startup-critical x/weight DMAs (first use is
        # the first _den, ~40us in).
        nc, p = self.nc, self.p
        sel_stg = p["consts"].tile([2, 128], f32)
        nc.sync.dma_start(sel_stg[:], self.selc_d[0:2, :])
        sel = p["consts"].tile([2, 128], bf16)
        nc.vector.tensor_copy(sel[:], sel_stg[:])
        id2 = p["consts"].tile([66, 2], f32)
        nc.sync.dma_start(id2[64:66, :], self.selc_d[2:4, 0:2])
        self.sel, self.id2 = sel, id2

    def _weights_qkv(self):
        nc, p = self.nc, self.p
        self.w_sb = {}
        for name in ("q", "k", "v"):
            w_sb = p["wpool"].tile([128, D], bf16, tag="w_" + name)
            nc.sync.dma_start(w_sb[:], self.w_d[name])
            self.w_sb[name] = w_sb

    def _weights_wo(self):
        nc, p = self.nc, self.p
        self.wo_sb = p["wpool"].tile([128, D], bf16, tag="w_o")
        nc.sync.dma_start(self.wo_sb[:], self.wo_d)

    def _st(self, b):
        return self.st_b.setdefault(b, {})

    def _xt_dma(self, b, qts=range(QT)):
        nc, p = self.nc, self.p
        st = self._st(b)
        if "xt" not in st:
            st["xt"] = p["xtp"].tile([128, 8 * S], bf16, tag="xt",
                                     name="xt")
        xt = st["xt"]
        # one contiguous dma_start per qt so the first projection group
        # of this batch only waits for its own slice set
        for qt in qts:
            nc.sync.dma_start(
                xt[:, qt * 4096:(qt + 1) * 4096],
                self.xtr_d[:, (b * QT + qt) * 4096:
                           (b * QT + qt + 1) * 4096])

    def _drain_fillers(self, n=None):
        while self.fillers and (n is None or n > 0):
            self.fillers.popleft()()
            self.n_drained += 1
            if n is not None:
                n -= 1

    def _drain_to(self, watermark):
        while self.n_drained < watermark and self.fillers:
            self.fillers.popleft()()
            self.n_drained += 1

    def _enq(self, thunk):
        self.fillers.append(thunk)
        self.n_enq += 1

    def _enqueue_proj(self, b, qt):
        nc, p = self.nc, self.p
        st = self._st(b)
        if "qT" not in st:
            st["qT"] = p["qkv"].tile([128, S], bf16, tag="qT", name="qT")
            st["kT"] = p["qkv"].tile([128, S], bf16, tag="kT", name="kT")
            st["vT"] = p["vtpool"].tile([128, S], bf16, tag="vT", name="vT")
        xt = st["xt"]
        # q first: the next q-tile's first scores matmul blocks on the qT
        # PSUM->SBUF cast, so its copy should clear DVE earliest
        for name in ("q", "k", "v"):
            dst = st[{"q": "qT", "k": "kT", "v": "vT"}[name]]
            if name == "v" and "v_tiles" not in st:
                st["v_tiles"] = [None] * KB
            box = {}

            def mk_mm(dc, name=name, box=box, qt=qt, xt=xt):
                def thunk():
                    if dc == 0:
                        box["pps"] = p["ps_a"].tile(
                            [128, 512], f32, tag="ps_a", name="pps")
                    nc.tensor.matmul(
                        box["pps"][:],
                        self.w_sb[name][:, dc * 128:(dc + 1) * 128],
                        xt[:, (qt * 8 + dc) * 512:(qt * 8 + dc + 1) * 512],
                        start=(dc == 0), stop=(dc == 7))
                return thunk

            for dc in range(8):
                self._enq(mk_mm(dc))

            def cp(dst=dst, box=box, qt=qt):
                nc.vector.tensor_copy(
                    dst[:, qt * 512:(qt + 1) * 512], box["pps"][:])

            self._enq(cp)
            if name == "v":
                for kb in range(4 * qt, 4 * qt + 4):
                    self._enq(
                        lambda kb=kb, b=b: self._vtrans_one(b, kb))

    def _enqueue_wo(self, b, qt):
        nc, p = self.nc, self.p
        aT = self._st(b)["aT"]
        box = {}
        for i, qb in enumerate(range(4 * qt, 4 * qt + 4)):
            def thunk(i=i, qb=qb, aT=aT, b=b, qt=qt, box=box):
                if i == 0:
                    box["o_sb"] = p["outp"].tile(
                        [128, 4096], bf16, tag="osb", name="osb")
                o_sb = box["o_sb"]
                for nt in range(2):
                    pout = p["ps_a"].tile([128, 512], f32, tag="ps_a",
                                          name="pout")
                    nc.tensor.matmul(
                        pout[:], aT[:, qb * 128:(qb + 1) * 128],
                        self.wo_sb[:, nt * 512:(nt + 1) * 512],
                        start=True, stop=True)
                    if (qb + nt) % 2 == 0:
                        nc.vector.tensor_copy(
                            o_sb[:, i * 1024 + nt * 512:
                                 i * 1024 + (nt + 1) * 512], pout[:])
                    else:
                        nc.scalar.copy(
                            o_sb[:, i * 1024 + nt * 512:
                                 i * 1024 + (nt + 1) * 512], pout[:])
                last = (b == B - 1 and qt == QT - 1)
                if last:
                    # tail: fire each qb row-block as soon as it's ready
                    nc.sync.dma_start(
                        self.out_d[b * S + qb * 128:
                                   b * S + (qb + 1) * 128, :],
                        o_sb[:, i * 1024:(i + 1) * 1024])
                elif i == 3:
                    rows = slice(b * S + qt * 512, b * S + (qt + 1) * 512)
                    nc.sync.dma_start(
                        self.out_d[rows, :].rearrange(
                            "(q p) c -> p q c", p=128),
                        o_sb[:].rearrange("p (q c) -> p q c", q=4))
            self._enq(thunk)

    def _proj_group(self, b, qt):
        nc, p = self.nc, self.p
        st = self._st(b)
        if "qT" not in st:
            st["qT"] = p["qkv"].tile([128, S], bf16, tag="qT", name="qT")
            st["kT"] = p["qkv"].tile([128, S], bf16, tag="kT", name="kT")
            st["vT"] = p["vtpool"].tile([128, S], bf16, tag="vT", name="vT")
        xt = st["xt"]
        for name, dst in (("q", st["qT"]), ("k", st["kT"]), ("v", st["vT"])):
            pps = p["ps_a"].tile([128, 512], f32, tag="ps_a")
            for dc in range(8):
                nc.tensor.matmul(
                    pps[:], self.w_sb[name][:, dc * 128:(dc + 1) * 128],
                    xt[:, (qt * 8 + dc) * 512:(qt * 8 + dc + 1) * 512],
                    start=(dc == 0), stop=(dc == 7))
            nc.vector.tensor_copy(dst[:, qt * 512:(qt + 1) * 512], pps[:])

    def _vtrans(self, b):
        st = self._st(b)
        st.setdefault("v_tiles", [None] * KB)
        for kb in range(KB):
            self._vtrans_one(b, kb)

    def _vtrans_one(self, b, kb):
        nc, p = self.nc, self.p
        st = self._st(b)
        vT = st["vT"]
        tp2f = p["ps_a"].tile([128, 512], f32, tag="ps_a")
        tp2 = tp2f[:, 0:64].bitcast(bf16)
        nc.tensor.transpose(
            tp2, vT[:, kb * 128:(kb + 1) * 128], self.ident[:])
        vt = p["vpp"].tile([128, 131], bf16, tag="vp")
        # [V_A(0:64) | 1(64) | V_B(65:129) | pad(129, unread) | 1(130)]
        nc.gpsimd.memset(vt[:, 64:65], 1.0)
        nc.gpsimd.memset(vt[:, 130:131], 1.0)
        nc.vector.tensor_copy(vt[:, 0:64], tp2[:, 0:64])
        nc.vector.tensor_copy(vt[:, 65:129], tp2[:, 64:128])
        st["v_tiles"][kb] = vt

    def _attn_prologue(self, b, qt):
        """Emit scores(0)+exp(0) of (b, qt) — called BEFORE the previous
        q-tile's den chain so the PE has attention work while the den's
        DVE copies / tiny transposes resolve at the boundary."""
        helpers = self._attn_helpers(b, qt)
        scores, exp = helpers
        stp, off = scores(0)
        pt = exp(0, stp, off)
        return helpers, stp, off, pt

    def _attention_qtile(self, b, qt, pro=None):
        nc, p = self.nc, self.p
        st = self._st(b)
        v_tiles = st["v_tiles"]
        if "aT" not in st:
            st["aT"] = p["atp"].tile([128, S], bf16, tag="aT", name="aT")
        if pro is None:
            pro = self._attn_prologue(b, qt)
        (scores, exp), stp, off, pt = pro
        pvA = p["ps_pv"].tile([128, 512], f32, tag="ps_pv")
        pvB = p["ps_pv"].tile([128, 512], f32, tag="ps_pv")
        st["pv"] = (pvA, pvB)
        nkb = 4 * qt + 4
        for kb in range(nkb):
            cur_pt, cur_off = pt, off
            if kb + 1 < nkb:
                stp, off = scores(kb + 1)
            self._drain_fillers(4 if kb % 2 == 0 else 3)
            nc.tensor.matmul(
                pvA[0:65, cur_off:512], v_tiles[kb][:, 0:65],
                cur_pt[:, cur_off:512],
                start=(kb == 0), stop=(kb == nkb - 1))
            nc.tensor.matmul(
                pvB[0:66, cur_off:512], v_tiles[kb][:, 65:131],
                cur_pt[:, 512 + cur_off:1024],
                start=(kb == 0), stop=(kb == nkb - 1))
            if kb + 1 < nkb:
                pt = exp(kb + 1, stp, off)

    def _attn_helpers(self, b, qt):
        nc, p = self.nc, self.p
        st = self._st(b)
        qT, kT = st["qT"], st["kT"]

        def scores(kb):
            off = max(0, (kb - 4 * qt) * 128)
            stp = p["ps_st"].tile([128, 1024], f32, tag="ps_st",
                                  name="stp")
            nc.tensor.matmul(
                stp[:, off:512], kT[0:64, kb * 128:(kb + 1) * 128],
                qT[0:64, qt * 512 + off:(qt + 1) * 512],
                start=True, stop=True)
            nc.tensor.matmul(
                stp[:, 512 + off:1024],
                kT[64:128, kb * 128:(kb + 1) * 128],
                qT[64:128, qt * 512 + off:(qt + 1) * 512],
                start=True, stop=True)
            return stp, off

        def exp(kb, stp, off):
            pt = p["ptp"].tile([128, 1024], bf16, tag="pt", name="pt")
            st_v = stp[:].rearrange("p (h q) -> p h q", h=2)[:, :, off:512]
            pt_v = pt[:].rearrange("p (h q) -> p h q", h=2)[:, :, off:512]
            nc.scalar.activation(pt_v, st_v, EXP, scale=0.125)
            if kb - 4 * qt >= 0:
                # post-exp 0/1 triangle multiply on Pool (SBUF-only)
                for hoff in (0, 512):
                    blk = pt[:, hoff + off:hoff + off + 128]
                    nc.gpsimd.tensor_mul(blk, blk, self.trimask[:])
            return pt

        return scores, exp

    def _den_part1(self, b, qt):
        nc, p = self.nc, self.p
        st = self._st(b)
        pvA, pvB = st.pop("pv")
        # stage psum out^T -> SBUF (bf16) and den rows; repartition the
        # dens [2,512] rows to [128,8] with tiny PE transposes (no DMAs)
        # stgB on ACT (it queues safely behind the already-emitted exp(0)
        # of the next q-tile) so the pv PSUM banks release ~0.7us sooner:
        # the next q-tile's first PV matmul waits on exactly this
        stgA = p["denp"].tile([128, 512], bf16, tag="stgA")
        nc.vector.tensor_copy(stgA[0:64, :], pvA[0:64, :])
        stgB = p["denp"].tile([128, 512], bf16, tag="stgB")
        nc.scalar.copy(stgB[0:64, :], pvB[0:64, :])
        dens = p["denp"].tile([128, 512], f32, tag="dens")
        nc.vector.tensor_copy(dens[64:66, :], pvB[64:66, :])
        nc.vector.tensor_copy(dens[64:65, :], pvA[64:65, :])
        # give the PE filler work while the DVE den copies run — the
        # repartition transposes below block the PE FIFO head on them
        self._drain_fillers(2)
        tpd = p["ps_a"].tile([128, 512], f32, tag="ps_a")
        for qh in range(4):
            nc.tensor.transpose(
                tpd[:, 2 * qh:2 * qh + 2],
                dens[64:66, 128 * qh:128 * (qh + 1)], self.id2[64:66, :])
        densR = p["denp"].tile([128, 8], bf16, tag="densR")
        with nc.allow_low_precision(
                reason="softmax reciprocal scale, bf16 is plenty"):
            nc.vector.reciprocal(densR[:], tpd[:, 0:8])
        st.setdefault("den_pend", {})[qt] = (stgA, stgB, densR)

    def _den_part2(self, b, qt):
        nc, p = self.nc, self.p
        st = self._st(b)
        aT = st["aT"]
        stgA, stgB, densR = st["den_pend"].pop(qt)
        cols = slice(qt * 512, (qt + 1) * 512)
        # transpose the reciprocals back to [2, 512] rows at partition 0
        rbf = p["ps_a"].tile([128, 512], f32, tag="ps_a")
        for qh in range(4):
            nc.tensor.transpose(
                rbf[0:2, 64 * qh:64 * (qh + 1)].bitcast(bf16),
                densR[:, 2 * qh:2 * qh + 2], self.ident[:])
        recip_r = p["denp"].tile([2, 512], bf16, tag="recip_r")
        nc.scalar.copy(recip_r[:], rbf[0:2, 0:256].bitcast(bf16))
        r_ps = p["ps_a"].tile([128, 512], f32, tag="ps_a")
        nc.tensor.matmul(r_ps[:], self.sel[:], recip_r[:],
                         start=True, stop=True)
        nc.sync.dma_start(aT[64:128, cols], stgB[0:64, :])
        nc.vector.tensor_mul(aT[0:64, cols], stgA[0:64, :], r_ps[0:64, :])
        nc.vector.tensor_mul(aT[64:128, cols], aT[64:128, cols],
                             r_ps[64:128, :])


_PROGRAM_CACHE = {}


def _get_program():
    if "nc" not in _PROGRAM_CACHE:
        _PROGRAM_CACHE["nc"] = build_program()
    return _PROGRAM_CACHE["nc"]


def _wtile(W, sl, b16):
    """[D, 128] weight slice -> SBUF layout [p, c*128+m] = W[c*128+p, m]."""
    w = np.asarray(W, dtype=np.float32)[:, sl]
    return np.ascontiguousarray(
        w.reshape(8, 128, CLOC).transpose(1, 0, 2).reshape(128, D)
    ).astype(b16)


def make_in_maps(x, Wq, Wk, Wv, Wo):
    import ml_dtypes
    b16 = ml_dtypes.bfloat16
    xtr = np.ascontiguousarray(
        np.asarray(x, dtype=np.float32)
        .reshape(B, QT, 512, 8, 128)
        .transpose(4, 0, 1, 3, 2)
        .reshape(128, B * QT * 8 * 512)).astype(b16)
    sel_const = np.zeros((4, CLOC), dtype=np.float32)
    sel_const[0, 0:64] = 1.0
    sel_const[1, 64:128] = 1.0
    sel_const[2, 0] = 1.0
    sel_const[3, 1] = 1.0
    maps = []
    for c in range(NCORES):
        sl = slice(c * CLOC, (c + 1) * CLOC)
        maps.append({
            "xtr": xtr,
            "wq": _wtile(Wq, sl, b16),
            "wk": _wtile(Wk, sl, b16),
            "wv": _wtile(Wv, sl, b16),
            "wo": np.ascontiguousarray(
                np.asarray(Wo, dtype=np.float32)[sl, :]).astype(b16),
            "selc": sel_const,
        })
    return maps


def run(x, Wq, Wk, Wv, Wo, bo, trace=False, **kw):
    nc = _get_program()
    maps = make_in_maps(x, Wq, Wk, Wv, Wo)
    res = run_bass_kernel_spmd(nc, maps, core_ids=list(range(NCORES)),
                               trace=trace, **kw)
    acc = res.results[0]["out"].astype(np.float32)
    for c in range(1, NCORES):
        acc = acc + res.results[c]["out"].astype(np.float32)
    out = (acc + np.asarray(bo, dtype=np.float32)).reshape(B, S, D)
    return out, res


def kernel(x, Wq, Wk, Wv, Wo, bo):
    out, _ = run(x, Wq, Wk, Wv, Wo, bo, trace=False)
    return out


# revision 77
# speedup vs baseline: 1.0389x; 1.0389x over previous
"""Causal multi-head attention on 8 trn2 NeuronCores.

Sharding: head-parallel. Each core owns 2 of the 16 heads (128 of the 1024
channels) for all 4 batches. Per core:
  Q^T/K^T/V^T projections (local 128 channels) from x^T (host-transposed,
  a pure layout prep like the weight slicing); flash-style causal attention
  in score-transposed layout S^T[k, q]; softmax denominators ride along as a
  ones column appended to V (PV matmul M=65/66, den lands on its own PSUM
  partition); normalization is applied to A^T via a rank-2 "R" matmul built
  from the reciprocals; local Wo row-block matmul produces a full
  [8192, 1024] partial per core, summed (+bias) on host.

Optimizations vs the f32r baseline (470us -> ~350us on HW):
  - All heavy matmuls in bf16 (host quantizes x/W; PSUM->SBUF copies cast).
    The baseline's float32r matmuls ran the PE in its HIGH-power fp32 mode,
    which tripped hardware duty-cycle throttling (HAM k=4/8 for 38% of the
    run; bf16 halves that). bf16 also halves the x / out DMA bytes.
  - Causal masking for diagonal blocks is a post-exp multiply by a 0/1
    triangle on the (otherwise idle) Pool engine, SBUF-only (GPSIMD cannot
    touch PSUM), freeing ~60us of DVE time.
  - The softmax-reciprocal repartition uses tiny PE transposes ([2,512] den
    rows -> [128,8] -> reciprocal -> back) instead of 16 tiny DMAs per
    q-tile; the baseline's 256 DMAs cost ~650ns of serial sync-queue issue
    each (166us total).
  - x^T and the QKV weights are pre-tiled on the host so every DMA is one
    contiguous span per partition: one dma_start per (batch, q-tile) at
    ~128 descriptors instead of 8 dma_starts x 1024 descriptors. Output
    DMA is one per q-tile group. Sync-engine time: 307us -> ~40us.
  - The attention kb loop is software-pipelined one deep: scores(kb+1) and
    PE filler work are emitted between exp(kb) and PV(kb), so the FIFO PE
    queue holds runnable work while the ACT exp / Pool mask run (PV at the
    head otherwise head-of-line blocks, and every PE idle gap also drops
    the PE out of its full-clock p-state).
  - The filler queue (projections of batch b+1, Wo of batch b) backlogs
    across batches with watermark force-drains for the data deadlines, so
    the last batch's attention still has PE work to interleave.
Softmax skips the max-subtraction (scores are bounded; fp32 exp cannot
overflow) and folds the 1/sqrt(64) scale into the ACT exp. Off-diagonal
causal key blocks are skipped entirely; straddle blocks only compute/exp
their valid columns. Final: HW exec ~349-355us, rel err 4.4e-3 (gate 2e-2).
"""
import sys

sys.path.insert(0, "/opt/trn_rl_repo")

import numpy as np

import concourse.bass as bass
import concourse.tile as tile
from concourse import bacc, mybir
from concourse.bass_utils import run_bass_kernel_spmd

f32 = mybir.dt.float32
f32r = mybir.dt.float32r
bf16 = mybir.dt.bfloat16
EXP = mybir.ActivationFunctionType.Exp

B, S, D, H, HD = 4, 2048, 1024, 16, 64
NCORES = 8
CLOC = D // NCORES       # 128 local channels = 2 heads per core
BS = B * S               # 8192
QT = 4                   # q tiles of 512 per batch
KB = 16                  # k blocks of 128 per batch


def build_program():
    """Build + compile the per-core Bacc program (identical on all cores)."""
    nc = bacc.Bacc("TRN2", target_bir_lowering=False, debug=False)

    # x^T pre-tiled on host: [128 p, (b, qt, dc, s)] so each (b, qt) DMA
    # reads one contiguous 8KB span per partition (few descriptors)
    xtr_d = nc.dram_tensor("xtr", [128, B * QT * 8 * 512], bf16,
                           kind="ExternalInput").ap()
    # qkv weights pre-tiled on host into the SBUF layout [p, (c, m)]
    wq_d = nc.dram_tensor("wq", [128, D], bf16, kind="ExternalInput").ap()
    wk_d = nc.dram_tensor("wk", [128, D], bf16, kind="ExternalInput").ap()
    wv_d = nc.dram_tensor("wv", [128, D], bf16, kind="ExternalInput").ap()
    wo_d = nc.dram_tensor("wo", [CLOC, D], bf16, kind="ExternalInput").ap()
    selc_d = nc.dram_tensor("selc", [4, CLOC], f32, kind="ExternalInput").ap()
    out_d = nc.dram_tensor("out", [BS, D], bf16, kind="ExternalOutput").ap()

    with tile.TileContext(nc) as tc:
        _Builder(nc, tc, xtr_d, wq_d, wk_d, wv_d, wo_d, selc_d, out_d).build()
    nc.compile()
    return nc


class _Builder:
    def __init__(self, nc, tc, xtr_d, wq_d, wk_d, wv_d, wo_d, selc_d, out_d):
        self.nc = nc
        self.tc = tc
        self.xtr_d = xtr_d
        self.w_d = {"q": wq_d, "k": wk_d, "v": wv_d}
        self.wo_d = wo_d
        self.selc_d = selc_d
        self.out_d = out_d
        self.st_b = {}   # per-batch state: xt, qT, kT, vT, aT, v_tiles
        from collections import deque
        self.fillers = deque()
        self.n_enq = 0       # total fillers ever enqueued
        self.n_drained = 0   # total fillers ever drained
        self.markers = {}    # key -> n_enq watermark that must be drained

    def build(self):
        from contextlib import ExitStack

        nc, tc = self.nc, self.tc
        with ExitStack() as ctx:
            p = self.p = {}
            for name, bufs, space in (
                ("consts", 1, None), ("wpool", 1, None), ("xtp", 1, None),
                ("qkv", 2, None), ("vtpool", 1, None), ("vpp", 24, None),
                ("ptp", 5, None), ("atp", 2, None), ("denp", 3, None),
                ("outp", 2, None),
                ("ps_a", 2, "PSUM"), ("ps_st", 2, "PSUM"),
                ("ps_pv", 2, "PSUM"),
            ):
                kw = {"space": space} if space else {}
                p[name] = ctx.enter_context(
                    tc.tile_pool(name=name, bufs=bufs, **kw))

            self._consts()

            # ---- software pipeline across batches: proj(b+1)/Wo(b)
            # queue as PE "filler" thunks drained inside the attention
            # kb loop so the PE queue never idles on exp waits. The queue
            # deliberately backlogs across batches (no per-batch flush) so
            # the last batch's attention still has PE filler work; marker
            # force-drains enforce the data deadlines. ----
            # DMA issue order matters for startup latency (sync-queue
            # issue is serial): first q-tile of x, then the QKV weights,
            # then the rest of x, then Wo.
            self._xt_dma(0, qts=(0,))
            self._weights_qkv()
            self._xt_dma(0, qts=(1, 2, 3))
            self._weights_wo()
            self._consts_dma()
            for qt in range(QT):
                self._proj_group(0, qt)
            self._vtrans(0)
            # pend = previous q-tile awaiting its den chain; its den_part1
            # is emitted AFTER the next q-tile's scores(0)/exp(0) prologue
            # so the PE crosses the boundary with attention work in the
            # FIFO instead of den transposes blocked on DVE copies
            pend = None
            for b in range(B):
                if b + 1 < B:
                    self._xt_dma(b + 1)
                    for qt in range(QT):
                        self._enqueue_proj(b + 1, qt)
                        self.markers[("proj", b + 1, qt)] = self.n_enq
                for qt in range(QT):
                    if b >= 1:
                        self._drain_to(self.markers[("proj", b, qt)])
                    if b >= 2 and qt == 0:
                        self._drain_to(self.markers[("wo", b - 2)])
                    pro = self._attn_prologue(b, qt)
                    if pend is not None:
                        self._den_part1(*pend)
                    self._attention_qtile(b, qt, pro)
                    if pend is not None:
                        self._den_part2(*pend)
                        self._enqueue_wo(*pend)
                        if pend[1] == QT - 1:
                            self.markers[("wo", pend[0])] = self.n_enq
                    pend = (b, qt)
            self._den_part1(*pend)
            self._den_part2(*pend)
            self._enqueue_wo(*pend)
            self._drain_fillers()

    # ------------------------------------------------------------------
    def _consts(self):
        nc, p = self.nc, self.p
        # bf16 identity for the V PE-transpose
        ident = p["consts"].tile([128, 128], bf16)
        nc.gpsimd.memset(ident[:], 0.0)
        nc.gpsimd.affine_select(
            out=ident[:], in_=ident[:],
            compare_op=mybir.AluOpType.not_equal, fill=1.0, base=0,
            pattern=[[-1, 128]], channel_multiplier=1,
        )
        # 0/1 lower-triangle (valid where key p <= query col) for the
        # post-exp diagonal-block mask multiply on Pool
        trimask = p["consts"].tile([128, 128], bf16)
        nc.gpsimd.memset(trimask[:], 1.0)
        nc.gpsimd.affine_select(
            out=trimask[:], in_=trimask[:],
            compare_op=mybir.AluOpType.is_ge, fill=0.0, base=0,
            pattern=[[1, 128]], channel_multiplier=-1,
        )
        self.ident, self.trimask = ident, trimask

    def _consts_dma(self):
        # selc rows 0:2 = head-A/head-B one-hots; rows 2:4 = 2x2 identity.
        # Issued after the startup-critical x/weight DMAs (first use is
        # the first _den, ~40us in).
        nc, p = self.nc, self.p
        sel_stg = p["consts"].tile([2, 128], f32)
        nc.sync.dma_start(sel_stg[:], self.selc_d[0:2, :])
        sel = p["consts"].tile([2, 128], bf16)
        nc.vector.tensor_copy(sel[:], sel_stg[:])
        id2 = p["consts"].tile([66, 2], f32)
        nc.sync.dma_start(id2[64:66, :], self.selc_d[2:4, 0:2])
        self.sel, self.id2 = sel, id2

    def _weights_qkv(self):
        nc, p = self.nc, self.p
        self.w_sb = {}
        for name in ("q", "k", "v"):
            w_sb = p["wpool"].tile([128, D], bf16, tag="w_" + name)
            nc.sync.dma_start(w_sb[:], self.w_d[name])
            self.w_sb[name] = w_sb

    def _weights_wo(self):
        nc, p = self.nc, self.p
        self.wo_sb = p["wpool"].tile([128, D], bf16, tag="w_o")
        nc.sync.dma_start(self.wo_sb[:], self.wo_d)

    def _st(self, b):
        return self.st_b.setdefault(b, {})

    def _xt_dma(self, b, qts=range(QT)):
        nc, p = self.nc, self.p
        st = self._st(b)
        if "xt" not in st:
            st["xt"] = p["xtp"].tile([128, 8 * S], bf16, tag="xt",
                                     name="xt")
        xt = st["xt"]
        # one contiguous dma_start per qt so the first projection group
        # of this batch only waits for its own slice set
        for qt in qts:
            nc.sync.dma_start(
                xt[:, qt * 4096:(qt + 1) * 4096],
                self.xtr_d[:, (b * QT + qt) * 4096:
                           (b * QT + qt + 1) * 4096])

    def _drain_fillers(self, n=None):
        while self.fillers and (n is None or n > 0):
            self.fillers.popleft()()
            self.n_drained += 1
            if n is not None:
                n -= 1

    def _drain_to(self, watermark):
        while self.n_drained < watermark and self.fillers:
            self.fillers.popleft()()
            self.n_drained += 1

    def _enq(self, thunk):
        self.fillers.append(thunk)
        self.n_enq += 1

    def _enqueue_proj(self, b, qt):
        nc, p = self.nc, self.p
        st = self._st(b)
        if "qT" not in st:
            st["qT"] = p["qkv"].tile([128, S], bf16, tag="qT", name="qT")
            st["kT"] = p["qkv"].tile([128, S], bf16, tag="kT", name="kT")
            st["vT"] = p["vtpool"].tile([128, S], bf16, tag="vT", name="vT")
        xt = st["xt"]
        # q first: the next q-tile's first scores matmul blocks on the qT
        # PSUM->SBUF cast, so its copy should clear DVE earliest
        for name in ("q", "k", "v"):
            dst = st[{"q": "qT", "k": "kT", "v": "vT"}[name]]
            if name == "v" and "v_tiles" not in st:
                st["v_tiles"] = [None] * KB
            box = {}

            def mk_mm(dc, name=name, box=box, qt=qt, xt=xt):
                def thunk():
                    if dc == 0:
                        box["pps"] = p["ps_a"].tile(
                            [128, 512], f32, tag="ps_a", name="pps")
                    nc.tensor.matmul(
                        box["pps"][:],
                        self.w_sb[name][:, dc * 128:(dc + 1) * 128],
                        xt[:, (qt * 8 + dc) * 512:(qt * 8 + dc + 1) * 512],
                        start=(dc == 0), stop=(dc == 7))
                return thunk

            for dc in range(8):
                self._enq(mk_mm(dc))

            def cp(dst=dst, box=box, qt=qt):
                nc.vector.tensor_copy(
                    dst[:, qt * 512:(qt + 1) * 512], box["pps"][:])

            self._enq(cp)
            if name == "v":
                for kb in range(4 * qt, 4 * qt + 4):
                    self._enq(
                        lambda kb=kb, b=b: self._vtrans_one(b, kb))

    def _enqueue_wo(self, b, qt):
        nc, p = self.nc, self.p
        aT = self._st(b)["aT"]
        box = {}
        for i, qb in enumerate(range(4 * qt, 4 * qt + 4)):
            def thunk(i=i, qb=qb, aT=aT, b=b, qt=qt, box=box):
                if i == 0:
                    box["o_sb"] = p["outp"].tile(
                        [128, 4096], bf16, tag="osb", name="osb")
                o_sb = box["o_sb"]
                for nt in range(2):
                    pout = p["ps_a"].tile([128, 512], f32, tag="ps_a",
                                          name="pout")
                    nc.tensor.matmul(
                        pout[:], aT[:, qb * 128:(qb + 1) * 128],
                        self.wo_sb[:, nt * 512:(nt + 1) * 512],
                        start=True, stop=True)
                    if (qb + nt) % 2 == 0:
                        nc.vector.tensor_copy(
                            o_sb[:, i * 1024 + nt * 512:
                                 i * 1024 + (nt + 1) * 512], pout[:])
                    else:
                        nc.scalar.copy(
                            o_sb[:, i * 1024 + nt * 512:
                                 i * 1024 + (nt + 1) * 512], pout[:])
                last = (b == B - 1 and qt == QT - 1)
                if last:
                    # tail: fire each qb row-block as soon as it's ready
                    nc.sync.dma_start(
                        self.out_d[b * S + qb * 128:
                                   b * S + (qb + 1) * 128, :],
                        o_sb[:, i * 1024:(i + 1) * 1024])
                elif i == 3:
                    rows = slice(b * S + qt * 512, b * S + (qt + 1) * 512)
                    nc.sync.dma_start(
                        self.out_d[rows, :].rearrange(
                            "(q p) c -> p q c", p=128),
                        o_sb[:].rearrange("p (q c) -> p q c", q=4))
            self._enq(thunk)

    def _proj_group(self, b, qt):
        nc, p = self.nc, self.p
        st = self._st(b)
        if "qT" not in st:
            st["qT"] = p["qkv"].tile([128, S], bf16, tag="qT", name="qT")
            st["kT"] = p["qkv"].tile([128, S], bf16, tag="kT", name="kT")
            st["vT"] = p["vtpool"].tile([128, S], bf16, tag="vT", name="vT")
        xt = st["xt"]
        for name, dst in (("q", st["qT"]), ("k", st["kT"]), ("v", st["vT"])):
            pps = p["ps_a"].tile([128, 512], f32, tag="ps_a")
            for dc in range(8):
                nc.tensor.matmul(
                    pps[:], self.w_sb[name][:, dc * 128:(dc + 1) * 128],
                    xt[:, (qt * 8 + dc) * 512:(qt * 8 + dc + 1) * 512],
                    start=(dc == 0), stop=(dc == 7))
            nc.vector.tensor_copy(dst[:, qt * 512:(qt + 1) * 512], pps[:])

    def _vtrans(self, b):
        st = self._st(b)
        st.setdefault("v_tiles", [None] * KB)
        for kb in range(KB):
            self._vtrans_one(b, kb)

    def _vtrans_one(self, b, kb):
        nc, p = self.nc, self.p
        st = self._st(b)
        vT = st["vT"]
        tp2f = p["ps_a"].tile([128, 512], f32, tag="ps_a")
        tp2 = tp2f[:, 0:64].bitcast(bf16)
        nc.tensor.transpose(
            tp2, vT[:, kb * 128:(kb + 1) * 128], self.ident[:])
        vt = p["vpp"].tile([128, 131], bf16, tag="vp")
        # [V_A(0:64) | 1(64) | V_B(65:129) | pad(129, unread) | 1(130)]
        nc.gpsimd.memset(vt[:, 64:65], 1.0)
        nc.gpsimd.memset(vt[:, 130:131], 1.0)
        nc.vector.tensor_copy(vt[:, 0:64], tp2[:, 0:64])
        nc.vector.tensor_copy(vt[:, 65:129], tp2[:, 64:128])
        st["v_tiles"][kb] = vt

    def _attn_prologue(self, b, qt):
        """Emit scores(0)+exp(0) of (b, qt) — called BEFORE the previous
        q-tile's den chain so the PE has attention work while the den's
        DVE copies / tiny transposes resolve at the boundary."""
        helpers = self._attn_helpers(b, qt)
        scores, exp = helpers
        stp, off = scores(0)
        pt = exp(0, stp, off)
        return helpers, stp, off, pt

    def _attention_qtile(self, b, qt, pro=None):
        nc, p = self.nc, self.p
        st = self._st(b)
        v_tiles = st["v_tiles"]
        if "aT" not in st:
            st["aT"] = p["atp"].tile([128, S], bf16, tag="aT", name="aT")
        if pro is None:
            pro = self._attn_prologue(b, qt)
        (scores, exp), stp, off, pt = pro
        pvA = p["ps_pv"].tile([128, 512], f32, tag="ps_pv")
        pvB = p["ps_pv"].tile([128, 512], f32, tag="ps_pv")
        st["pv"] = (pvA, pvB)
        nkb = 4 * qt + 4
        for kb in range(nkb):
            cur_pt, cur_off = pt, off
            if kb + 1 < nkb:
                stp, off = scores(kb + 1)
            self._drain_fillers(4 if kb % 2 == 0 else 3)
            nc.tensor.matmul(
                pvA[0:65, cur_off:512], v_tiles[kb][:, 0:65],
                cur_pt[:, cur_off:512],
                start=(kb == 0), stop=(kb == nkb - 1))
            nc.tensor.matmul(
                pvB[0:66, cur_off:512], v_tiles[kb][:, 65:131],
                cur_pt[:, 512 + cur_off:1024],
                start=(kb == 0), stop=(kb == nkb - 1))
            if kb + 1 < nkb:
                pt = exp(kb + 1, stp, off)

    def _attn_helpers(self, b, qt):
        nc, p = self.nc, self.p
        st = self._st(b)
        qT, kT = st["qT"], st["kT"]

        def scores(kb):
            off = max(0, (kb - 4 * qt) * 128)
            stp = p["ps_st"].tile([128, 1024], f32, tag="ps_st",
                                  name="stp")
            nc.tensor.matmul(
                stp[:, off:512], kT[0:64, kb * 128:(kb + 1) * 128],
                qT[0:64, qt * 512 + off:(qt + 1) * 512],
                start=True, stop=True)
            nc.tensor.matmul(
                stp[:, 512 + off:1024],
                kT[64:128, kb * 128:(kb + 1) * 128],
                qT[64:128, qt * 512 + off:(qt + 1) * 512],
                start=True, stop=True)
            return stp, off

        def exp(kb, stp, off):
            pt = p["ptp"].tile([128, 1024], bf16, tag="pt", name="pt")
            st_v = stp[:].rearrange("p (h q) -> p h q", h=2)[:, :, off:512]
            pt_v = pt[:].rearrange("p (h q) -> p h q", h=2)[:, :, off:512]
            nc.scalar.activation(pt_v, st_v, EXP, scale=0.125)
            if kb - 4 * qt >= 0:
                # post-exp 0/1 triangle multiply on Pool (SBUF-only)
                for hoff in (0, 512):
                    blk = pt[:, hoff + off:hoff + off + 128]
                    nc.gpsimd.tensor_mul(blk, blk, self.trimask[:])
            return pt

        return scores, exp

    def _den_part1(self, b, qt):
        nc, p = self.nc, self.p
        st = self._st(b)
        pvA, pvB = st.pop("pv")
        # stage psum out^T -> SBUF (bf16) and den rows; repartition the
        # dens [2,512] rows to [128,8] with tiny PE transposes (no DMAs)
        stgA = p["denp"].tile([128, 512], bf16, tag="stgA")
        nc.vector.tensor_copy(stgA[0:64, :], pvA[0:64, :])
        stgB = p["denp"].tile([128, 512], bf16, tag="stgB")
        nc.vector.tensor_copy(stgB[0:64, :], pvB[0:64, :])
        dens = p["denp"].tile([128, 512], f32, tag="dens")
        nc.vector.tensor_copy(dens[64:66, :], pvB[64:66, :])
        nc.vector.tensor_copy(dens[64:65, :], pvA[64:65, :])
        # give the PE filler work while the DVE den copies run — the
        # repartition transposes below block the PE FIFO head on them
        self._drain_fillers(2)
        tpd = p["ps_a"].tile([128, 512], f32, tag="ps_a")
        for qh in range(4):
            nc.tensor.transpose(
                tpd[:, 2 * qh:2 * qh + 2],
                dens[64:66, 128 * qh:128 * (qh + 1)], self.id2[64:66, :])
        densR = p["denp"].tile([128, 8], bf16, tag="densR")
        with nc.allow_low_precision(
                reason="softmax reciprocal scale, bf16 is plenty"):
            nc.vector.reciprocal(densR[:], tpd[:, 0:8])
        st.setdefault("den_pend", {})[qt] = (stgA, stgB, densR)

    def _den_part2(self, b, qt):
        nc, p = self.nc, self.p
        st = self._st(b)
        aT = st["aT"]
        stgA, stgB, densR = st["den_pend"].pop(qt)
        cols = slice(qt * 512, (qt + 1) * 512)
        # transpose the reciprocals back to [2, 512] rows at partition 0
        rbf = p["ps_a"].tile([128, 512], f32, tag="ps_a")
        for qh in range(4):
            nc.tensor.transpose(
                rbf[0:2, 64 * qh:64 * (qh + 1)].bitcast(bf16),
                densR[:, 2 * qh:2 * qh + 2], self.ident[:])
        recip_r = p["denp"].tile([2, 512], bf16, tag="recip_r")
        nc.scalar.copy(recip_r[:], rbf[0:2, 0:256].bitcast(bf16))
        r_ps = p["ps_a"].tile([128, 512], f32, tag="ps_a")
        nc.tensor.matmul(r_ps[:], self.sel[:], recip_r[:],
                         start=True, stop=True)
        nc.sync.dma_start(aT[64:128, cols], stgB[0:64, :])
        nc.vector.tensor_mul(aT[0:64, cols], stgA[0:64, :], r_ps[0:64, :])
        nc.vector.tensor_mul(aT[64:128, cols], aT[64:128, cols],
                             r_ps[64:128, :])


_PROGRAM_CACHE = {}


def _get_program():
    if "nc" not in _PROGRAM_CACHE:
        _PROGRAM_CACHE["nc"] = build_program()
    return _PROGRAM_CACHE["nc"]


def _wtile(W, sl, b16):
    """[D, 128] weight slice -> SBUF layout [p, c*128+m] = W[c*128+p, m]."""
    w = np.asarray(W, dtype=np.float32)[:, sl]
    return np.ascontiguousarray(
        w.reshape(8, 128, CLOC).transpose(1, 0, 2).reshape(128, D)
    ).astype(b16)


def make_in_maps(x, Wq, Wk, Wv, Wo):
    import ml_dtypes
    b16 = ml_dtypes.bfloat16
    xtr = np.ascontiguousarray(
        np.asarray(x, dtype=np.float32)
        .reshape(B, QT, 512, 8, 128)
        .transpose(4, 0, 1, 3, 2)
        .reshape(128, B * QT * 8 * 512)).astype(b16)
    sel_const = np.zeros((4, CLOC), dtype=np.float32)
    sel_const[0, 0:64] = 1.0
    sel_const[1, 64:128] = 1.0
    sel_const[2, 0] = 1.0
    sel_const[3, 1] = 1.0
    maps = []
    for c in range(NCORES):
        sl = slice(c * CLOC, (c + 1) * CLOC)
        maps.append({
            "xtr": xtr,
            "wq": _wtile(Wq, sl, b16),
            "wk": _wtile(Wk, sl, b16),
            "wv": _wtile(Wv, sl, b16),
            "wo": np.ascontiguousarray(
                np.asarray(Wo, dtype=np.float32)[sl, :]).astype(b16),
            "selc": sel_const,
        })
    return maps


def run(x, Wq, Wk, Wv, Wo, bo, trace=False, **kw):
    nc = _get_program()
    maps = make_in_maps(x, Wq, Wk, Wv, Wo)
    res = run_bass_kernel_spmd(nc, maps, core_ids=list(range(NCORES)),
                               trace=trace, **kw)
    acc = res.results[0]["out"].astype(np.float32)
    for c in range(1, NCORES):
        acc = acc + res.results[c]["out"].astype(np.float32)
    out = (acc + np.asarray(bo, dtype=np.float32)).reshape(B, S, D)
    return out, res


def kernel(x, Wq, Wk, Wv, Wo, bo):
    out, _ = run(x, Wq, Wk, Wv, Wo, bo, trace=False)
    return out
